# revision 29
# baseline (speedup 1.0000x reference)
"""Trainium2 Bass kernel for nn_DA3CrossFrameRKDDistanceLoss.

Math (reference semantics):
  ref rows (teacher/student frame 0, ref_perm subsample), extra = teacher
  frames [1,3,5,7] concat -> [4096, D].  Cosine top-4 neighbours of each ref
  row inside extra;  KL(softmax(diff_t) || softmax(diff_s)) per row with
  diff pairs (d1: ref-shared, d2: ref-simhigh, d3: shared-simhigh), smooth-L1
  (beta=0.5) of each KL, averaged per branch and summed.

  kl = (sum(exp(a) * (a - b)) / Sa) - ln Sa + ln Sb   with a = diff_t,
  b = diff_s, Sa = sum(exp(a)), Sb = sum(exp(b)).  d2 in the reference loop
  is identical across the 3 shared frames, so it is computed once here.

Sharding: 8 cores = (batch b in 0..3) x (half h of the 256 ref rows).
Each core: fp32 matmul [128 refs x 4096 extras x 1024] (extT streamed in 8
chunks, 8 PSUM banks), top-4 via DVE max/max_index, sim_high row gathers
from DRAM via 4x gpsimd indirect_dma_start, KL pipeline on DVE (subs, muls)
and ACT (exp / copy with accumulate).  Per-core output: smooth-L1 values
[128 rows, 19 units]; host sums and divides by the branch counts (the
"all-reduce" of the scalar loss).

Build quirks for this container's walrus: at most ONE sync-wait encodes per
compute instruction, so _split_waits() rewrites the scheduled program,
moving extra waits onto injected same-engine Drain carriers; tensor_tensor_
reduce / scalar_tensor_tensor / gpsimd load_library fail codegen here, so
the fused mul+reduce is DVE tensor_mul + ACT Copy(accum_out), and gathers
avoid the library-gated dma_gather.
"""

import os
import sys

import numpy as np

for _p in ("/opt/trn_rl_repo", "/root/.axon_site/_ro/trn_rl_repo"):
    # later inserts go to the front: prefer the axon-site copy when present
    if os.path.isdir(_p) and _p not in sys.path:
        sys.path.insert(0, _p)

import concourse.bass as bass
import concourse.tile as tile
from concourse import library_config, mybir
from concourse.bass_utils import run_bass_kernel_spmd

F32 = mybir.dt.float32
I16 = mybir.dt.int16
U16 = mybir.dt.uint16

B = 4
P = 1024
D = 1024
NUM_REF = 256
TOPK = 4
NREF_CORE = 128          # ref rows per core
NEXTRA = 4 * P           # 4096
EXTRA_FRAMES = (1, 3, 5, 7)
SHARED_T = (2, 4, 6)
SHARED_S = (1, 2, 3)
NFRAMES = 3
# unit columns: 3 d1 + 4 d2 + 12 d3
N_UNITS = 19
N_CHUNK = 8              # sim free-dim chunks of 512
CHUNK = NEXTRA // N_CHUNK
KT = D // 128            # 8 contraction tiles

ALU = mybir.AluOpType
ACTF = mybir.ActivationFunctionType

_BUILT = None


def _split_waits(nc):
    """Walrus in this container encodes at most one sync-wait per compute
    instruction. Split extras onto same-engine Drain carriers placed just
    before (engines execute in program order, so semantics are identical)."""
    ctr = [0]

    def process(block):
        new = []
        for inst in block.instructions:
            si = inst.sync_info
            waits = list(si.on_wait) if si is not None and si.on_wait else []
            if len(waits) > 1:
                for w in waits[:-1]:
                    ctr[0] += 1
                    nop = mybir.InstDrain(
                        name=f"waitnop-{ctr[0]}",
                        engine=inst.engine,
                        ins=[],
                        outs=[],
                        sync_info=mybir.SyncInfo(on_wait=[w], on_update=[]),
                    )
                    new.append(nop)
                inst.sync_info = mybir.SyncInfo(
                    on_wait=[waits[-1]], on_update=list(si.on_update or [])
                )
            new.append(inst)
        block.instructions = new
        for b in getattr(block, "blocks", []) or []:
            process(b)

    for b in nc.m.functions[0].blocks:
        process(b)


def _build_module():
    """Trace the per-core Bass program (identical on all 8 cores)."""
    nc = bass.Bass()

    refT = nc.declare_dram_parameter("refT", [D, NREF_CORE], F32, isOutput=False)
    extT = nc.declare_dram_parameter("extT", [D, NEXTRA], F32, isOutput=False)
    extnat = nc.declare_dram_parameter("extnat", [NEXTRA, D], F32, isOutput=False)
    # packed [rt, rs, st0, st1, st2, ss0, ss1, ss2] — one DMA, one sem lane
    sm_d = nc.declare_dram_parameter("small", [8, NREF_CORE, D], F32, isOutput=False)
    hub_d = nc.declare_dram_parameter("hub", [NREF_CORE, N_UNITS], F32, isOutput=True)

    with tile.TileContext(nc) as tc:
        with (
            tc.tile_pool(name="singles", bufs=1) as singles,
            tc.tile_pool(name="ext", bufs=3) as ext,
            tc.tile_pool(name="ps", bufs=8, space="PSUM") as ps,
            tc.tile_pool(name="ab", bufs=2) as ab,
        ):
            dma = nc.sync.dma_start

            # ---- resident small inputs -------------------------------------
            sm = singles.tile([NREF_CORE, 8, D], F32)
            refT_sb = singles.tile([128, KT, 128], F32)
            dma(out=refT_sb, in_=refT.ap().rearrange("(k p) m -> p k m", p=128))
            rt = sm[:, 0, :]
            rs = sm[:, 1, :]
            st = sm[:, 2:5, :]
            ss = sm[:, 5:8, :]

            sim = singles.tile([128, NEXTRA], F32)
            simhigh = singles.tile([128, TOPK, D], F32)
            cand = singles.tile([128, N_CHUNK * 8], F32)
            topv = singles.tile([128, 8], F32)
            topi = singles.tile([128, 8], U16)

            SaAll = singles.tile([128, N_UNITS], F32)
            SbAll = singles.tile([128, N_UNITS], F32)
            SAll = singles.tile([128, N_UNITS], F32)
            SyAll = singles.tile([128, NFRAMES], F32)

            c2 = singles.tile([NREF_CORE, D], F32)
            c3 = singles.tile([NREF_CORE, NFRAMES, D], F32)
            absorb = singles.tile([128, 1], F32)

            def kl_unit(x1, x2, y1, y2, cvec, col, sy=None):
                """Accumulate Sa/Sb/S (and Sy) for one KL unit.

                a = x1 - x2, b = y1 - y2;  Sa/Sb = sum(exp());  S col gets
                sum(exp(a) * cvec)  (cvec == a - b), or sum(exp(a) * a) with
                sy getting sum(exp(a) * b) when cvec is None.
                """
                a = ab.tile([128, D], F32, tag="a")
                nc.vector.tensor_sub(a, x1, x2)
                ea = ab.tile([128, D], F32, tag="ea")
                nc.scalar.activation(
                    ea, a, ACTF.Exp, accum_out=SaAll[:, col : col + 1]
                )
                b = ab.tile([128, D], F32, tag="b")
                nc.vector.tensor_sub(b, y1, y2)
                eb = ab.tile([128, D], F32, tag="eb")
                nc.scalar.activation(
                    eb, b, ACTF.Exp, accum_out=SbAll[:, col : col + 1]
                )
                # tiny DVE read of ea: absorbs the ACT->DVE wait so the mult
                # below carries at most one sync wait (walrus 1-wait limit)
                nc.vector.tensor_copy(absorb, ea[:, :1])

                # prod = ea * c on DVE;  sum via ACT copy-with-accumulate
                def mul_accum(x, acc):
                    prod = ab.tile([128, D], F32, tag="prod")
                    nc.vector.tensor_mul(prod, ea, x)
                    trash = ab.tile([128, D], F32, tag="trash")
                    nc.scalar.activation(
                        trash, prod, ACTF.Copy, accum_out=acc
                    )

                if cvec is not None:
                    mul_accum(cvec, SAll[:, col : col + 1])
                else:
                    mul_accum(a, SAll[:, col : col + 1])
                    mul_accum(b, sy)

            # ---- similarity matmul + per-chunk top8 ------------------------
            # all 8 chunks get their own PSUM bank — no cross-chunk WAR waits
            pts = [
                ps.tile([128, CHUNK], F32, tag="pt", name=f"pt{c}")
                for c in range(N_CHUNK)
            ]
            # dummy matmul reading only refT: absorbs the refT-DMA wait so the
            # first real matmul carries only the extT-chunk wait
            nc.tensor.matmul(
                pts[0][:, :8], lhsT=refT_sb[:, 0, :], rhs=refT_sb[:, 0, :8],
                start=True, stop=True,
            )
            extT_re = extT.ap().rearrange("(k p) m -> p k m", p=128)
            for c in range(N_CHUNK):
                et = ext.tile([128, KT, CHUNK], F32)
                dma(out=et, in_=extT_re[:, :, c * CHUNK : (c + 1) * CHUNK])
                pt = pts[c]
                for k in range(KT):
                    nc.tensor.matmul(
                        pt, lhsT=refT_sb[:, k, :], rhs=et[:, k, :],
                        start=(k == 0), stop=(k == KT - 1),
                    )
                nc.scalar.copy(sim[:, c * CHUNK : (c + 1) * CHUNK], pt)
                nc.vector.max(
                    cand[:, c * 8 : (c + 1) * 8],
                    sim[:, c * CHUNK : (c + 1) * CHUNK],
                )

            # small input lands after the extT stream — d1/d2/d3 need it
            # only after the matmul phase
            nc.scalar.dma_start(out=sm, in_=sm_d.ap().rearrange("t n d -> n t d"))

            # ---- global top4 + row gathers ---------------------------------
            nc.vector.max(topv, cand)
            nc.vector.max_index(topi, topv, sim)
            topi32 = singles.tile([128, TOPK], mybir.dt.int32)
            nc.vector.tensor_copy(topi32, topi[:, :TOPK])
            # per-j indirect row gather from DRAM: simhigh[:, j, :] =
            # extnat[topi[:, j], :]  (standard InstDMACopy on qPoolDynamic)
            for j in range(TOPK):
                nc.gpsimd.indirect_dma_start(
                    out=simhigh[:, j, :],
                    out_offset=None,
                    in_=extnat.ap(),
                    in_offset=bass.IndirectOffsetOnAxis(
                        ap=topi32[:, j : j + 1], axis=0
                    ),
                )

            # ---- d1 + c2/c3: independent of the gathers — fills the
            # topk/gather latency gap on DVE/ACT
            nc.vector.tensor_sub(c2, rt, rs)
            for f in range(NFRAMES):
                nc.vector.tensor_sub(c3[:, f, :], st[:, f, :], ss[:, f, :])
            for f in range(NFRAMES):
                kl_unit(
                    rt, st[:, f, :], rs, ss[:, f, :], None, f,
                    sy=SyAll[:, f : f + 1],
                )

            # ---- d2 / d3 ---------------------------------------------------
            for j in range(TOPK):
                sh = simhigh[:, j, :]
                kl_unit(rt, sh, rs, sh, c2, 3 + j)
            for f in range(NFRAMES):
                for j in range(TOPK):
                    sh = simhigh[:, j, :]
                    kl_unit(
                        st[:, f, :], sh, ss[:, f, :], sh, c3[:, f, :],
                        7 + f * TOPK + j,
                    )

            # ---- tail: kl, smooth-l1, writeback ----------------------------
            nc.vector.tensor_sub(SAll[:, 0:NFRAMES], SAll[:, 0:NFRAMES], SyAll)
            recip = singles.tile([128, N_UNITS], F32)
            nc.vector.reciprocal(recip, SaAll)
            kl = singles.tile([128, N_UNITS], F32)
            nc.vector.tensor_mul(kl, SAll, recip)
            lnsa = singles.tile([128, N_UNITS], F32)
            nc.scalar.activation(lnsa, SaAll, ACTF.Ln)
            lnsb = singles.tile([128, N_UNITS], F32)
            nc.scalar.activation(lnsb, SbAll, ACTF.Ln)
            nc.vector.tensor_sub(kl, kl, lnsa)
            nc.vector.tensor_add(kl, kl, lnsb)

            kl2 = singles.tile([128, N_UNITS], F32)
            nc.vector.tensor_mul(kl2, kl, kl)
            km = singles.tile([128, N_UNITS], F32)
            nc.vector.tensor_scalar(km, kl, 0.25, None, op0=ALU.subtract)
            mask = singles.tile([128, N_UNITS], mybir.dt.uint8)
            nc.vector.tensor_scalar(mask, kl, 0.5, None, op0=ALU.is_lt)
            hub = singles.tile([128, N_UNITS], F32)
            nc.vector.select(hub, mask, kl2, km)
            dma(out=hub_d.ap(), in_=hub)

    _split_waits(nc)
    return nc


def get_module():
    global _BUILT
    if _BUILT is None:
        _BUILT = _build_module()
    return _BUILT


def make_in_maps(teacher_feats, student_feats, ref_perm, shared_perm):
    """Host-side sharding: slice/normalize/layout the per-core inputs."""
    tf = np.ascontiguousarray(np.asarray(teacher_feats, dtype=np.float32))
    sf = np.ascontiguousarray(np.asarray(student_feats, dtype=np.float32))
    rp = np.asarray(ref_perm, dtype=np.int64)
    sp = np.asarray(shared_perm, dtype=np.int64)[: NUM_REF]

    in_maps = []
    for b in range(B):
        extra = np.ascontiguousarray(
            tf[b, list(EXTRA_FRAMES)].reshape(NEXTRA, D)
        )
        en = np.maximum(
            np.sqrt((extra.astype(np.float32) ** 2).sum(axis=1)), 1e-12
        ).astype(np.float32)
        extT = np.ascontiguousarray((extra / en[:, None]).T)
        ref_t = tf[b, 0][rp]                      # [256, D] raw
        ref_s = sf[b, 0][rp]
        rn = np.maximum(
            np.sqrt((ref_t.astype(np.float32) ** 2).sum(axis=1, keepdims=True)),
            1e-12,
        ).astype(np.float32)
        refn = ref_t / rn
        st_all = np.stack([tf[b, t][sp] for t in SHARED_T])   # [3, 256, D]
        ss_all = np.stack([sf[b, s][sp] for s in SHARED_S])
        for h in range(2):
            sl = slice(h * NREF_CORE, (h + 1) * NREF_CORE)
            small = np.concatenate(
                [ref_t[None, sl], ref_s[None, sl], st_all[:, sl], ss_all[:, sl]],
                axis=0,
            )
            in_maps.append(
                dict(
                    refT=np.ascontiguousarray(refn[sl].T),
                    extT=extT,
                    extnat=extra,
                    small=np.ascontiguousarray(small),
                )
            )
    return in_maps


def finish(hub_stack):
    """hub_stack: [8, 128, 19] per-core smooth-l1 values -> scalar loss."""
    hs = np.asarray(hub_stack, dtype=np.float64)
    d1 = hs[..., 0:3].sum()
    d2 = hs[..., 3:7].sum()
    d3 = hs[..., 7:19].sum()
    n_d1 = NFRAMES * B * NUM_REF                 # 3072
    n_d2 = B * NUM_REF * TOPK                    # 4096 (dedup: loop adds 3x)
    n_d3 = NFRAMES * B * NUM_REF * TOPK          # 12288
    return np.float32(d1 / n_d1 + d2 / n_d2 + d3 / n_d3)


def run(in_maps, trace=False):
    nc = get_module()
    res = run_bass_kernel_spmd(nc, in_maps, list(range(8)), trace=trace)
    return res


def kernel(teacher_feats, student_feats, ref_perm, shared_perm):
    in_maps = make_in_maps(teacher_feats, student_feats, ref_perm, shared_perm)
    res = run(in_maps)
    hub = np.stack([r["hub"] for r in res.results])
    return finish(hub)


# revision 31
# speedup vs baseline: 1.1100x; 1.1100x over previous
"""Trainium2 Bass kernel for nn_DA3CrossFrameRKDDistanceLoss.

Math (reference semantics):
  ref rows (teacher/student frame 0, ref_perm subsample), extra = teacher
  frames [1,3,5,7] concat -> [4096, D].  Cosine top-4 neighbours of each ref
  row inside extra;  KL(softmax(diff_t) || softmax(diff_s)) per row with
  diff pairs (d1: ref-shared, d2: ref-simhigh, d3: shared-simhigh), smooth-L1
  (beta=0.5) of each KL, averaged per branch and summed.

  kl = (sum(exp(a) * (a - b)) / Sa) - ln Sa + ln Sb   with a = diff_t,
  b = diff_s, Sa = sum(exp(a)), Sb = sum(exp(b)).  d2 in the reference loop
  is identical across the 3 shared frames, so it is computed once here.

Sharding: 8 cores = (batch b in 0..3) x (half h of the 256 ref rows).
Each core: fp32 matmul [128 refs x 4096 extras x 1024] (extT streamed in 8
chunks, 8 PSUM banks), top-4 via DVE max/max_index, sim_high row gathers
from DRAM via 4x gpsimd indirect_dma_start, KL pipeline on DVE (subs, muls)
and ACT (exp / copy with accumulate).  Per-core output: smooth-L1 values
[128 rows, 19 units]; host sums and divides by the branch counts (the
"all-reduce" of the scalar loss).

Build quirks for this container's walrus: at most ONE sync-wait encodes per
compute instruction, so _split_waits() rewrites the scheduled program,
moving extra waits onto injected same-engine Drain carriers; tensor_tensor_
reduce / scalar_tensor_tensor / gpsimd load_library fail codegen here, so
the fused mul+reduce is DVE tensor_mul + ACT Copy(accum_out), and gathers
avoid the library-gated dma_gather.
"""

import os
import sys

import numpy as np

for _p in ("/opt/trn_rl_repo", "/root/.axon_site/_ro/trn_rl_repo"):
    # later inserts go to the front: prefer the axon-site copy when present
    if os.path.isdir(_p) and _p not in sys.path:
        sys.path.insert(0, _p)

import concourse.bass as bass
import concourse.tile as tile
from concourse import library_config, mybir
from concourse.bass_utils import run_bass_kernel_spmd

F32 = mybir.dt.float32
BF16 = mybir.dt.bfloat16
I16 = mybir.dt.int16
U16 = mybir.dt.uint16

B = 4
P = 1024
D = 1024
NUM_REF = 256
TOPK = 4
NREF_CORE = 128          # ref rows per core
NEXTRA = 4 * P           # 4096
EXTRA_FRAMES = (1, 3, 5, 7)
SHARED_T = (2, 4, 6)
SHARED_S = (1, 2, 3)
NFRAMES = 3
# unit columns: 3 d1 + 4 d2 + 12 d3
N_UNITS = 19
N_CHUNK = 8              # sim free-dim chunks of 512
CHUNK = NEXTRA // N_CHUNK
KT = D // 128            # 8 contraction tiles

ALU = mybir.AluOpType
ACTF = mybir.ActivationFunctionType

_BUILT = None


def _split_waits(nc):
    """Walrus in this container encodes at most one sync-wait per compute
    instruction. Split extras onto same-engine Drain carriers placed just
    before (engines execute in program order, so semantics are identical)."""
    ctr = [0]

    def process(block):
        new = []
        for inst in block.instructions:
            si = inst.sync_info
            waits = list(si.on_wait) if si is not None and si.on_wait else []
            if len(waits) > 1:
                for w in waits[:-1]:
                    ctr[0] += 1
                    nop = mybir.InstDrain(
                        name=f"waitnop-{ctr[0]}",
                        engine=inst.engine,
                        ins=[],
                        outs=[],
                        sync_info=mybir.SyncInfo(on_wait=[w], on_update=[]),
                    )
                    new.append(nop)
                inst.sync_info = mybir.SyncInfo(
                    on_wait=[waits[-1]], on_update=list(si.on_update or [])
                )
            new.append(inst)
        block.instructions = new
        for b in getattr(block, "blocks", []) or []:
            process(b)

    for b in nc.m.functions[0].blocks:
        process(b)


def _build_module():
    """Trace the per-core Bass program (identical on all 8 cores)."""
    nc = bass.Bass()

    refT = nc.declare_dram_parameter("refT", [D, NREF_CORE], F32, isOutput=False)
    extT = nc.declare_dram_parameter("extT", [D, NEXTRA], F32, isOutput=False)
    extnat = nc.declare_dram_parameter("extnat", [NEXTRA, D], F32, isOutput=False)
    # packed [rt, rs, st0, st1, st2, ss0, ss1, ss2] — one DMA, one sem lane
    sm_d = nc.declare_dram_parameter("small", [8, NREF_CORE, D], BF16, isOutput=False)
    hub_d = nc.declare_dram_parameter("hub", [NREF_CORE, N_UNITS], F32, isOutput=True)

    with tile.TileContext(nc) as tc:
        with (
            tc.tile_pool(name="singles", bufs=1) as singles,
            tc.tile_pool(name="ext", bufs=3) as ext,
            tc.tile_pool(name="ps", bufs=8, space="PSUM") as ps,
            tc.tile_pool(name="ab", bufs=2) as ab,
        ):
            dma = nc.sync.dma_start

            # ---- resident small inputs -------------------------------------
            sm = singles.tile([NREF_CORE, 8, D], BF16)
            refT_sb = singles.tile([128, KT, 128], F32)
            dma(out=refT_sb, in_=refT.ap().rearrange("(k p) m -> p k m", p=128))
            rt = sm[:, 0, :]
            rs = sm[:, 1, :]
            st = sm[:, 2:5, :]
            ss = sm[:, 5:8, :]

            sim = singles.tile([128, NEXTRA], F32)
            simhigh = singles.tile([128, TOPK, D], F32)
            cand = singles.tile([128, N_CHUNK * 8], F32)
            topv = singles.tile([128, 8], F32)
            topi = singles.tile([128, 8], U16)

            SaAll = singles.tile([128, N_UNITS], F32)
            SbAll = singles.tile([128, N_UNITS], F32)
            SAll = singles.tile([128, N_UNITS], F32)
            SyAll = singles.tile([128, NFRAMES], F32)

            c2 = singles.tile([NREF_CORE, D], BF16)
            c3 = singles.tile([NREF_CORE, NFRAMES, D], BF16)
            absorb = singles.tile([128, 1], BF16)
            redctr = [0]

            def kl_unit(x1, x2, y1, y2, cvec, col, sy=None):
                """Accumulate Sa/Sb/S (and Sy) for one KL unit.

                a = x1 - x2, b = y1 - y2;  Sa/Sb = sum(exp());  S col gets
                sum(exp(a) * cvec)  (cvec == a - b), or sum(exp(a) * a) with
                sy getting sum(exp(a) * b) when cvec is None.
                """
                a = ab.tile([128, D], BF16, tag="a")
                nc.vector.tensor_sub(a, x1, x2)
                ea = ab.tile([128, D], BF16, tag="ea")
                nc.scalar.activation(
                    ea, a, ACTF.Exp, accum_out=SaAll[:, col : col + 1]
                )
                b = ab.tile([128, D], BF16, tag="b")
                nc.vector.tensor_sub(b, y1, y2)
                eb = ab.tile([128, D], BF16, tag="eb")
                nc.scalar.activation(
                    eb, b, ACTF.Exp, accum_out=SbAll[:, col : col + 1]
                )
                # tiny DVE read of ea: absorbs the ACT->DVE wait so the mult
                # below carries at most one sync wait (walrus 1-wait limit)
                nc.vector.tensor_copy(absorb, ea[:, :1])

                # prod = ea * c on DVE; sum on DVE or ACT (balance engines)
                def mul_accum(x, acc):
                    prod = ab.tile([128, D], BF16, tag="prod")
                    nc.vector.tensor_mul(prod, ea, x)
                    redctr[0] += 1
                    if redctr[0] % 3 == 0:
                        trash = ab.tile([128, D], BF16, tag="trash")
                        nc.scalar.activation(
                            trash, prod, ACTF.Copy, accum_out=acc
                        )
                    else:
                        nc.vector.reduce_sum(
                            acc, prod, axis=mybir.AxisListType.X
                        )

                if cvec is not None:
                    mul_accum(cvec, SAll[:, col : col + 1])
                else:
                    mul_accum(a, SAll[:, col : col + 1])
                    mul_accum(b, sy)

            # ---- similarity matmul + per-chunk top8 ------------------------
            # all 8 chunks get their own PSUM bank — no cross-chunk WAR waits
            pts = [
                ps.tile([128, CHUNK], F32, tag="pt", name=f"pt{c}")
                for c in range(N_CHUNK)
            ]
            # dummy matmul reading only refT: absorbs the refT-DMA wait so the
            # first real matmul carries only the extT-chunk wait
            nc.tensor.matmul(
                pts[0][:, :8], lhsT=refT_sb[:, 0, :], rhs=refT_sb[:, 0, :8],
                start=True, stop=True,
            )
            extT_re = extT.ap().rearrange("(k p) m -> p k m", p=128)
            for c in range(N_CHUNK):
                et = ext.tile([128, KT, CHUNK], F32)
                dma(out=et, in_=extT_re[:, :, c * CHUNK : (c + 1) * CHUNK])
                pt = pts[c]
                for k in range(KT):
                    nc.tensor.matmul(
                        pt, lhsT=refT_sb[:, k, :], rhs=et[:, k, :],
                        start=(k == 0), stop=(k == KT - 1),
                    )
                nc.scalar.copy(sim[:, c * CHUNK : (c + 1) * CHUNK], pt)
                nc.vector.max(
                    cand[:, c * 8 : (c + 1) * 8],
                    sim[:, c * CHUNK : (c + 1) * CHUNK],
                )

            # small input lands after the extT stream — d1/d2/d3 need it
            # only after the matmul phase
            nc.scalar.dma_start(out=sm, in_=sm_d.ap().rearrange("t n d -> n t d"))

            # ---- global top4 + row gathers ---------------------------------
            nc.vector.max(topv, cand)
            nc.vector.max_index(topi, topv, sim)
            topi32 = singles.tile([128, TOPK], mybir.dt.int32)
            nc.vector.tensor_copy(topi32, topi[:, :TOPK])
            # per-j indirect row gather from DRAM: simhigh[:, j, :] =
            # extnat[topi[:, j], :]  (standard InstDMACopy on qPoolDynamic)
            for j in range(TOPK):
                nc.gpsimd.indirect_dma_start(
                    out=simhigh[:, j, :],
                    out_offset=None,
                    in_=extnat.ap(),
                    in_offset=bass.IndirectOffsetOnAxis(
                        ap=topi32[:, j : j + 1], axis=0
                    ),
                )

            # ---- d1 + c2/c3: independent of the gathers — fills the
            # topk/gather latency gap on DVE/ACT
            nc.vector.tensor_sub(c2, rt, rs)
            for f in range(NFRAMES):
                nc.vector.tensor_sub(c3[:, f, :], st[:, f, :], ss[:, f, :])
            for f in range(NFRAMES):
                kl_unit(
                    rt, st[:, f, :], rs, ss[:, f, :], None, f,
                    sy=SyAll[:, f : f + 1],
                )

            # bf16 copy of gathered rows: keeps the whole KL tail in 2x mode
            shb = singles.tile([128, TOPK, D], BF16)
            for j in range(TOPK):
                nc.vector.tensor_copy(shb[:, j, :], simhigh[:, j, :])

            # ---- d2 / d3 ---------------------------------------------------
            for j in range(TOPK):
                sh = shb[:, j, :]
                kl_unit(rt, sh, rs, sh, c2, 3 + j)
            for f in range(NFRAMES):
                for j in range(TOPK):
                    sh = shb[:, j, :]
                    kl_unit(
                        st[:, f, :], sh, ss[:, f, :], sh, c3[:, f, :],
                        7 + f * TOPK + j,
                    )

            # ---- tail: kl, smooth-l1, writeback ----------------------------
            nc.vector.tensor_sub(SAll[:, 0:NFRAMES], SAll[:, 0:NFRAMES], SyAll)
            recip = singles.tile([128, N_UNITS], F32)
            nc.vector.reciprocal(recip, SaAll)
            kl = singles.tile([128, N_UNITS], F32)
            nc.vector.tensor_mul(kl, SAll, recip)
            lnsa = singles.tile([128, N_UNITS], F32)
            nc.scalar.activation(lnsa, SaAll, ACTF.Ln)
            lnsb = singles.tile([128, N_UNITS], F32)
            nc.scalar.activation(lnsb, SbAll, ACTF.Ln)
            nc.vector.tensor_sub(kl, kl, lnsa)
            nc.vector.tensor_add(kl, kl, lnsb)

            kl2 = singles.tile([128, N_UNITS], F32)
            nc.vector.tensor_mul(kl2, kl, kl)
            km = singles.tile([128, N_UNITS], F32)
            nc.vector.tensor_scalar(km, kl, 0.25, None, op0=ALU.subtract)
            mask = singles.tile([128, N_UNITS], mybir.dt.uint8)
            nc.vector.tensor_scalar(mask, kl, 0.5, None, op0=ALU.is_lt)
            hub = singles.tile([128, N_UNITS], F32)
            nc.vector.select(hub, mask, kl2, km)
            dma(out=hub_d.ap(), in_=hub)

    _split_waits(nc)
    return nc


def get_module():
    global _BUILT
    if _BUILT is None:
        _BUILT = _build_module()
    return _BUILT


def make_in_maps(teacher_feats, student_feats, ref_perm, shared_perm):
    """Host-side sharding: slice/normalize/layout the per-core inputs."""
    tf = np.ascontiguousarray(np.asarray(teacher_feats, dtype=np.float32))
    sf = np.ascontiguousarray(np.asarray(student_feats, dtype=np.float32))
    rp = np.asarray(ref_perm, dtype=np.int64)
    sp = np.asarray(shared_perm, dtype=np.int64)[: NUM_REF]

    in_maps = []
    for b in range(B):
        extra = np.ascontiguousarray(
            tf[b, list(EXTRA_FRAMES)].reshape(NEXTRA, D)
        )
        en = np.maximum(
            np.sqrt((extra.astype(np.float32) ** 2).sum(axis=1)), 1e-12
        ).astype(np.float32)
        extT = np.ascontiguousarray((extra / en[:, None]).T)
        ref_t = tf[b, 0][rp]                      # [256, D] raw
        ref_s = sf[b, 0][rp]
        rn = np.maximum(
            np.sqrt((ref_t.astype(np.float32) ** 2).sum(axis=1, keepdims=True)),
            1e-12,
        ).astype(np.float32)
        refn = ref_t / rn
        st_all = np.stack([tf[b, t][sp] for t in SHARED_T])   # [3, 256, D]
        ss_all = np.stack([sf[b, s][sp] for s in SHARED_S])
        for h in range(2):
            sl = slice(h * NREF_CORE, (h + 1) * NREF_CORE)
            import ml_dtypes
            small = np.concatenate(
                [ref_t[None, sl], ref_s[None, sl], st_all[:, sl], ss_all[:, sl]],
                axis=0,
            ).astype(ml_dtypes.bfloat16)
            in_maps.append(
                dict(
                    refT=np.ascontiguousarray(refn[sl].T),
                    extT=extT,
                    extnat=extra,
                    small=np.ascontiguousarray(small),
                )
            )
    return in_maps


def finish(hub_stack):
    """hub_stack: [8, 128, 19] per-core smooth-l1 values -> scalar loss."""
    hs = np.asarray(hub_stack, dtype=np.float64)
    d1 = hs[..., 0:3].sum()
    d2 = hs[..., 3:7].sum()
    d3 = hs[..., 7:19].sum()
    n_d1 = NFRAMES * B * NUM_REF                 # 3072
    n_d2 = B * NUM_REF * TOPK                    # 4096 (dedup: loop adds 3x)
    n_d3 = NFRAMES * B * NUM_REF * TOPK          # 12288
    return np.float32(d1 / n_d1 + d2 / n_d2 + d3 / n_d3)


def run(in_maps, trace=False):
    nc = get_module()
    res = run_bass_kernel_spmd(nc, in_maps, list(range(8)), trace=trace)
    return res


def kernel(teacher_feats, student_feats, ref_perm, shared_perm):
    in_maps = make_in_maps(teacher_feats, student_feats, ref_perm, shared_perm)
    res = run(in_maps)
    hub = np.stack([r["hub"] for r in res.results])
    return finish(hub)


# revision 36
# speedup vs baseline: 1.3446x; 1.2114x over previous
"""Trainium2 Bass kernel for nn_DA3CrossFrameRKDDistanceLoss.

Math (reference semantics):
  ref rows (teacher/student frame 0, ref_perm subsample), extra = teacher
  frames [1,3,5,7] concat -> [4096, D].  Cosine top-4 neighbours of each ref
  row inside extra;  KL(softmax(diff_t) || softmax(diff_s)) per row with
  diff pairs (d1: ref-shared, d2: ref-simhigh, d3: shared-simhigh), smooth-L1
  (beta=0.5) of each KL, averaged per branch and summed.

  kl = (sum(exp(a) * (a - b)) / Sa) - ln Sa + ln Sb   with a = diff_t,
  b = diff_s, Sa = sum(exp(a)), Sb = sum(exp(b)).  d2 in the reference loop
  is identical across the 3 shared frames, so it is computed once here.

Sharding: 8 cores = (batch b in 0..3) x (half h of the 256 ref rows).
Each core: fp32 matmul [128 refs x 4096 extras x 1024] (extT streamed in 8
chunks, 8 PSUM banks), top-4 via DVE max/max_index, sim_high row gathers
from DRAM via 4x gpsimd indirect_dma_start, KL pipeline on DVE (subs, muls)
and ACT (exp / copy with accumulate).  Per-core output: smooth-L1 values
[128 rows, 19 units]; host sums and divides by the branch counts (the
"all-reduce" of the scalar loss).

Build quirks for this container's walrus: at most ONE sync-wait encodes per
compute instruction, so _split_waits() rewrites the scheduled program,
moving extra waits onto injected same-engine Drain carriers; tensor_tensor_
reduce / scalar_tensor_tensor / gpsimd load_library fail codegen here, so
the fused mul+reduce is DVE tensor_mul + ACT Copy(accum_out), and gathers
avoid the library-gated dma_gather.
"""

import os
import sys

import numpy as np

for _p in ("/opt/trn_rl_repo", "/root/.axon_site/_ro/trn_rl_repo"):
    # later inserts go to the front: prefer the axon-site copy when present
    if os.path.isdir(_p) and _p not in sys.path:
        sys.path.insert(0, _p)

import concourse.bass as bass
import concourse.tile as tile
from concourse import library_config, mybir
from concourse.bass_utils import run_bass_kernel_spmd

F32 = mybir.dt.float32
F32R = mybir.dt.float32r
BF16 = mybir.dt.bfloat16
I16 = mybir.dt.int16
U16 = mybir.dt.uint16

B = 4
P = 1024
D = 1024
NUM_REF = 256
TOPK = 4
NREF_CORE = 128          # ref rows per core
NEXTRA = 4 * P           # 4096
EXTRA_FRAMES = (1, 3, 5, 7)
SHARED_T = (2, 4, 6)
SHARED_S = (1, 2, 3)
NFRAMES = 3
# unit columns: 3 d1 + 4 d2 + 12 d3
N_UNITS = 19
N_CHUNK = 8              # sim free-dim chunks of 512
CHUNK = NEXTRA // N_CHUNK
KT = D // 128            # 8 contraction tiles

ALU = mybir.AluOpType
ACTF = mybir.ActivationFunctionType

_BUILT = None


def _split_waits(nc):
    """Walrus in this container encodes at most one sync-wait per compute
    instruction. Split extras onto same-engine Drain carriers placed just
    before (engines execute in program order, so semantics are identical)."""
    ctr = [0]

    def process(block):
        new = []
        for inst in block.instructions:
            si = inst.sync_info
            waits = list(si.on_wait) if si is not None and si.on_wait else []
            if len(waits) > 1:
                for w in waits[:-1]:
                    ctr[0] += 1
                    nop = mybir.InstDrain(
                        name=f"waitnop-{ctr[0]}",
                        engine=inst.engine,
                        ins=[],
                        outs=[],
                        sync_info=mybir.SyncInfo(on_wait=[w], on_update=[]),
                    )
                    new.append(nop)
                inst.sync_info = mybir.SyncInfo(
                    on_wait=[waits[-1]], on_update=list(si.on_update or [])
                )
            new.append(inst)
        block.instructions = new
        for b in getattr(block, "blocks", []) or []:
            process(b)

    for b in nc.m.functions[0].blocks:
        process(b)


def _build_module():
    """Trace the per-core Bass program (identical on all 8 cores)."""
    nc = bass.Bass()

    refT = nc.declare_dram_parameter("refT", [D, NREF_CORE], F32R, isOutput=False)
    extT = nc.declare_dram_parameter("extT", [D, NEXTRA], F32R, isOutput=False)
    extnat = nc.declare_dram_parameter("extnat", [NEXTRA, D], F32, isOutput=False)
    # packed [rt, rs, st0, st1, st2, ss0, ss1, ss2] — one DMA, one sem lane
    sm_d = nc.declare_dram_parameter("small", [8, NREF_CORE, D], BF16, isOutput=False)
    hub_d = nc.declare_dram_parameter("hub", [NREF_CORE, N_UNITS], F32, isOutput=True)

    with tile.TileContext(nc) as tc:
        with (
            tc.tile_pool(name="singles", bufs=1) as singles,
            tc.tile_pool(name="ext", bufs=3) as ext,
            tc.tile_pool(name="ps", bufs=8, space="PSUM") as ps,
            tc.tile_pool(name="ab", bufs=2) as ab,
        ):
            dma = nc.sync.dma_start

            # ---- resident small inputs -------------------------------------
            sm = singles.tile([NREF_CORE, 8, D], BF16)
            refT_sb = singles.tile([128, KT, 128], F32R)
            dma(out=refT_sb, in_=refT.ap().rearrange("(k p) m -> p k m", p=128))
            rt = sm[:, 0, :]
            rs = sm[:, 1, :]
            st = sm[:, 2:5, :]
            ss = sm[:, 5:8, :]

            sim = singles.tile([128, NEXTRA], F32)
            simhigh = singles.tile([128, TOPK, D], F32)
            cand = singles.tile([128, N_CHUNK * 8], F32)
            topv = singles.tile([128, 8], F32)
            topi = singles.tile([128, 8], U16)

            SaAll = singles.tile([128, N_UNITS], F32)
            SbAll = singles.tile([128, N_UNITS], F32)
            SAll = singles.tile([128, N_UNITS], F32)
            SyAll = singles.tile([128, NFRAMES], F32)

            c2 = singles.tile([NREF_CORE, D], BF16)
            c3 = singles.tile([NREF_CORE, NFRAMES, D], BF16)
            absorb = singles.tile([128, 1], BF16)
            redctr = [0]

            def kl_unit(x1, x2, y1, y2, cvec, col, sy=None):
                """Accumulate Sa/Sb/S (and Sy) for one KL unit.

                a = x1 - x2, b = y1 - y2;  Sa/Sb = sum(exp());  S col gets
                sum(exp(a) * cvec)  (cvec == a - b), or sum(exp(a) * a) with
                sy getting sum(exp(a) * b) when cvec is None.
                """
                a = ab.tile([128, D], BF16, tag="a")
                nc.vector.tensor_sub(a, x1, x2)
                ea = ab.tile([128, D], BF16, tag="ea")
                nc.scalar.activation(
                    ea, a, ACTF.Exp, accum_out=SaAll[:, col : col + 1]
                )
                b = ab.tile([128, D], BF16, tag="b")
                nc.vector.tensor_sub(b, y1, y2)
                eb = ab.tile([128, D], BF16, tag="eb")
                nc.scalar.activation(
                    eb, b, ACTF.Exp, accum_out=SbAll[:, col : col + 1]
                )
                # tiny DVE read of ea: absorbs the ACT->DVE wait so the mult
                # below carries at most one sync wait (walrus 1-wait limit)
                nc.vector.tensor_copy(absorb, ea[:, :1])

                # prod = ea * c on DVE; sum on DVE or ACT (balance engines)
                def mul_accum(x, acc):
                    prod = ab.tile([128, D], BF16, tag="prod")
                    nc.vector.tensor_mul(prod, ea, x)
                    redctr[0] += 1
                    if redctr[0] % 3 == 0:
                        trash = ab.tile([128, D], BF16, tag="trash")
                        nc.scalar.activation(
                            trash, prod, ACTF.Copy, accum_out=acc
                        )
                    else:
                        nc.vector.reduce_sum(
                            acc, prod, axis=mybir.AxisListType.X
                        )

                if cvec is not None:
                    mul_accum(cvec, SAll[:, col : col + 1])
                else:
                    mul_accum(a, SAll[:, col : col + 1])
                    mul_accum(b, sy)

            # ---- similarity matmul + per-chunk top8 ------------------------
            # all 8 chunks get their own PSUM bank — no cross-chunk WAR waits
            pts = [
                ps.tile([128, CHUNK], F32, tag="pt", name=f"pt{c}")
                for c in range(N_CHUNK)
            ]
            # dummy matmul reading only refT: absorbs the refT-DMA wait so the
            # first real matmul carries only the extT-chunk wait
            nc.tensor.matmul(
                pts[0][:, :8], lhsT=refT_sb[:, 0, :], rhs=refT_sb[:, 0, :8],
                start=True, stop=True,
            )
            extT_re = extT.ap().rearrange("(k p) m -> p k m", p=128)
            for c in range(N_CHUNK):
                et = ext.tile([128, KT, CHUNK], F32R)
                dma(out=et, in_=extT_re[:, :, c * CHUNK : (c + 1) * CHUNK])
                pt = pts[c]
                for k in range(KT):
                    nc.tensor.matmul(
                        pt, lhsT=refT_sb[:, k, :], rhs=et[:, k, :],
                        start=(k == 0), stop=(k == KT - 1),
                    )
                nc.scalar.copy(sim[:, c * CHUNK : (c + 1) * CHUNK], pt)
                nc.vector.max(
                    cand[:, c * 8 : (c + 1) * 8],
                    sim[:, c * CHUNK : (c + 1) * CHUNK],
                )

            # small input lands after the extT stream — d1/d2/d3 need it
            # only after the matmul phase
            nc.scalar.dma_start(out=sm, in_=sm_d.ap().rearrange("t n d -> n t d"))

            # ---- global top4 + row gathers ---------------------------------
            nc.vector.max(topv, cand)
            nc.vector.max_index(topi, topv, sim)
            topi32 = singles.tile([128, TOPK], mybir.dt.int32)
            nc.vector.tensor_copy(topi32, topi[:, :TOPK])
            # per-j indirect row gather from DRAM: simhigh[:, j, :] =
            # extnat[topi[:, j], :]  (standard InstDMACopy on qPoolDynamic)
            for j in range(TOPK):
                nc.gpsimd.indirect_dma_start(
                    out=simhigh[:, j, :],
                    out_offset=None,
                    in_=extnat.ap(),
                    in_offset=bass.IndirectOffsetOnAxis(
                        ap=topi32[:, j : j + 1], axis=0
                    ),
                )

            # ---- d1 + c2/c3: independent of the gathers — fills the
            # topk/gather latency gap on DVE/ACT
            nc.vector.tensor_sub(c2, rt, rs)
            for f in range(NFRAMES):
                nc.vector.tensor_sub(c3[:, f, :], st[:, f, :], ss[:, f, :])
            for f in range(NFRAMES):
                kl_unit(
                    rt, st[:, f, :], rs, ss[:, f, :], None, f,
                    sy=SyAll[:, f : f + 1],
                )

            # bf16 copy of gathered rows: keeps the whole KL tail in 2x mode
            shb = singles.tile([128, TOPK, D], BF16)
            for j in range(TOPK):
                nc.vector.tensor_copy(shb[:, j, :], simhigh[:, j, :])

            # ---- d2 / d3 ---------------------------------------------------
            for j in range(TOPK):
                sh = shb[:, j, :]
                kl_unit(rt, sh, rs, sh, c2, 3 + j)
            for f in range(NFRAMES):
                for j in range(TOPK):
                    sh = shb[:, j, :]
                    kl_unit(
                        st[:, f, :], sh, ss[:, f, :], sh, c3[:, f, :],
                        7 + f * TOPK + j,
                    )

            # ---- tail: kl, smooth-l1, writeback ----------------------------
            nc.vector.tensor_sub(SAll[:, 0:NFRAMES], SAll[:, 0:NFRAMES], SyAll)
            recip = singles.tile([128, N_UNITS], F32)
            nc.vector.reciprocal(recip, SaAll)
            kl = singles.tile([128, N_UNITS], F32)
            nc.vector.tensor_mul(kl, SAll, recip)
            lnsa = singles.tile([128, N_UNITS], F32)
            nc.scalar.activation(lnsa, SaAll, ACTF.Ln)
            lnsb = singles.tile([128, N_UNITS], F32)
            nc.scalar.activation(lnsb, SbAll, ACTF.Ln)
            nc.vector.tensor_sub(kl, kl, lnsa)
            nc.vector.tensor_add(kl, kl, lnsb)

            kl2 = singles.tile([128, N_UNITS], F32)
            nc.vector.tensor_mul(kl2, kl, kl)
            km = singles.tile([128, N_UNITS], F32)
            nc.vector.tensor_scalar(km, kl, 0.25, None, op0=ALU.subtract)
            mask = singles.tile([128, N_UNITS], mybir.dt.uint8)
            nc.vector.tensor_scalar(mask, kl, 0.5, None, op0=ALU.is_lt)
            hub = singles.tile([128, N_UNITS], F32)
            nc.vector.select(hub, mask, kl2, km)
            dma(out=hub_d.ap(), in_=hub)

    _split_waits(nc)
    return nc


def get_module():
    global _BUILT
    if _BUILT is None:
        _BUILT = _build_module()
    return _BUILT


def make_in_maps(teacher_feats, student_feats, ref_perm, shared_perm):
    """Host-side sharding: slice/normalize/layout the per-core inputs."""
    tf = np.ascontiguousarray(np.asarray(teacher_feats, dtype=np.float32))
    sf = np.ascontiguousarray(np.asarray(student_feats, dtype=np.float32))
    rp = np.asarray(ref_perm, dtype=np.int64)
    sp = np.asarray(shared_perm, dtype=np.int64)[: NUM_REF]

    in_maps = []
    for b in range(B):
        extra = np.ascontiguousarray(
            tf[b, list(EXTRA_FRAMES)].reshape(NEXTRA, D)
        )
        en = np.maximum(
            np.sqrt((extra.astype(np.float32) ** 2).sum(axis=1)), 1e-12
        ).astype(np.float32)
        extT = np.ascontiguousarray((extra / en[:, None]).T)
        ref_t = tf[b, 0][rp]                      # [256, D] raw
        ref_s = sf[b, 0][rp]
        rn = np.maximum(
            np.sqrt((ref_t.astype(np.float32) ** 2).sum(axis=1, keepdims=True)),
            1e-12,
        ).astype(np.float32)
        refn = ref_t / rn
        st_all = np.stack([tf[b, t][sp] for t in SHARED_T])   # [3, 256, D]
        ss_all = np.stack([sf[b, s][sp] for s in SHARED_S])
        for h in range(2):
            sl = slice(h * NREF_CORE, (h + 1) * NREF_CORE)
            import ml_dtypes
            small = np.concatenate(
                [ref_t[None, sl], ref_s[None, sl], st_all[:, sl], ss_all[:, sl]],
                axis=0,
            ).astype(ml_dtypes.bfloat16)
            in_maps.append(
                dict(
                    refT=np.ascontiguousarray(refn[sl].T),
                    extT=extT,
                    extnat=extra,
                    small=np.ascontiguousarray(small),
                )
            )
    return in_maps


def finish(hub_stack):
    """hub_stack: [8, 128, 19] per-core smooth-l1 values -> scalar loss."""
    hs = np.asarray(hub_stack, dtype=np.float64)
    d1 = hs[..., 0:3].sum()
    d2 = hs[..., 3:7].sum()
    d3 = hs[..., 7:19].sum()
    n_d1 = NFRAMES * B * NUM_REF                 # 3072
    n_d2 = B * NUM_REF * TOPK                    # 4096 (dedup: loop adds 3x)
    n_d3 = NFRAMES * B * NUM_REF * TOPK          # 12288
    return np.float32(d1 / n_d1 + d2 / n_d2 + d3 / n_d3)


def run(in_maps, trace=False):
    nc = get_module()
    res = run_bass_kernel_spmd(nc, in_maps, list(range(8)), trace=trace)
    return res


def kernel(teacher_feats, student_feats, ref_perm, shared_perm):
    in_maps = make_in_maps(teacher_feats, student_feats, ref_perm, shared_perm)
    res = run(in_maps)
    hub = np.stack([r["hub"] for r in res.results])
    return finish(hub)
